# revision 1
# baseline (speedup 1.0000x reference)
"""Masked multi-head attention block (B=8, N=1024, D=768, H=12) on 8 NeuronCores.

Strategy: pure data-parallel over batch (1 batch element per core).  Per core,
the whole attention block runs in a transpose-free dataflow:

  phase 1a: qkT[e, n]  = WqkvT(lhsT) @ xT        (q,k in [head_dim, seq] layout)
  phase 1b: v[n, e]    = xT(lhsT) @ WvT          (v in natural [seq, head_dim] layout,
                                                  stored interleaved with a ones column)
  phase 2:  ST[j, i]   = kT(lhsT, K=64) @ qT     (scores TRANSPOSED: softmax axis on
                                                  partitions; head pairs run concurrently
                                                  in the two 64-row halves of the PE array)
            P = exp(ST*scale + key_mask_bias)    (ACT, per-partition bias kills masked keys)
            P[i,i] += (1-m_i)*1e15               (diag add; makes padded-query columns
                                                  one-hot after normalization, to fp32
                                                  precision, since G dominates the sums)
  phase 3:  OT'[d+1, i] = Vaug(lhsT) @ P         (ones column of Vaug yields the softmax
                                                  denominator Z as row 64 for free)
            R = 1/Z (recip_approx_fast), Rb = ones x R  (PE K=1 broadcast matmul, fp32)
            otn = OT'[0:64] * Rb                 (normalized attn output, transposed)
  phase 4:  out[n, e]  = otn(lhsT) @ WprojT + ones(K=1) x bproj

All big matmuls run in float32r (full PE rate at moving-dim >= 256; ~2e-4 relative).
Input DMAs are consolidated and spread across the three DMA-capable queues
(sync / scalar / gpsimd) so descriptor generation does not serialize the prologue.
"""
import sys
for _p in ('/opt/trn_rl_repo',):
    if _p not in sys.path:
        sys.path.insert(0, _p)

from contextlib import ExitStack

import numpy as np

import concourse.bass as bass
import concourse.bacc as bacc
import concourse.mybir as mybir
import concourse.tile as tile
from concourse import bass_utils

F32 = mybir.dt.float32
F32R = mybir.dt.float32r
AF = mybir.ActivationFunctionType

B, N, D, H, HD = 8, 1024, 768, 12, 64
P = 128
DT = D // P            # 6 d-tiles
SCALE = HD ** -0.5
NEGMASK = -30000.0     # exp(x + NEGMASK) == 0.0 in fp32 for any realistic score
BIGG = 1e15            # diagonal dominance constant for padded-query rows


def build_nc(n=N, debug=False):
    NT = n // P                    # seq tiles (8)
    CH = min(512, n)               # matmul moving-dim chunk
    NCH = n // CH                  # chunks (2)

    nc = bacc.Bacc("TRN2", target_bir_lowering=False, debug=False)

    xT_d = nc.dram_tensor("xT", [P, DT * n], F32, kind="ExternalInput")
    wqkvT_d = nc.dram_tensor("wqkvT", [P, DT * 3 * D], F32, kind="ExternalInput")
    wprojT_d = nc.dram_tensor("wprojT", [P, DT * D], F32, kind="ExternalInput")
    bproj_d = nc.dram_tensor("bproj", [1, D], F32, kind="ExternalInput")
    mbias_d = nc.dram_tensor("mbias", [P, NT], F32, kind="ExternalInput")
    omm_d = nc.dram_tensor("omm", [P, NT], F32, kind="ExternalInput")
    ones_d = nc.dram_tensor("onesv", [1, P], F32, kind="ExternalInput")
    out_d = nc.dram_tensor("out", [n, D], F32, kind="ExternalOutput")

    def rr(ap):
        return ap.bitcast(F32R)

    with tile.TileContext(nc) as tc, ExitStack() as ctx:
        persist = ctx.enter_context(tc.tile_pool(name="persist", bufs=1))
        qk = persist.tile([P, 2 * DT, n], F32R)       # e-tiles: 0..5 = q, 6..11 = k
        vaug = persist.tile([P, NT, H, HD + 1], F32R)  # v natural + ones column
        otn = persist.tile([P, DT, n], F32R)          # normalized attn out, transposed
        dtl = persist.tile([P, NT, P], F32R)          # diag((1-m)*G) blocks
        mb = persist.tile([P, NT], F32)
        om = persist.tile([P, NT], F32R)
        ones = persist.tile([1, P], F32R)
        ones_f = persist.tile([1, P], F32)
        bpj = persist.tile([1, D], F32R)

        nc.sync.dma_start(mb, mbias_d.ap())
        nc.sync.dma_start(om, rr(omm_d.ap()))
        nc.sync.dma_start(ones, rr(ones_d.ap()))
        nc.sync.dma_start(ones_f, ones_d.ap())
        nc.sync.dma_start(bpj, rr(bproj_d.ap()))
        # ones column of vaug via gpsimd partition broadcast (memset cannot
        # write f32r, and a zero-step broadcast DMA degenerates to 4B packets)
        nc.gpsimd.partition_broadcast(
            vaug[:, :, :, HD].rearrange("p a b -> p (a b)"),
            ones[0:1, 0:1].to_broadcast((1, NT * H)), channels=P)
        for t in range(NT):
            nc.gpsimd.affine_select(
                out=dtl[:, t, :],
                in_=om[:, t:t + 1].to_broadcast((P, P)),
                pattern=[[-1, P]],
                compare_op=mybir.AluOpType.is_equal,
                fill=0.0, base=0, channel_multiplier=1,
            )

        # ---------------- phase 1: projections ----------------
        with tc.tile_pool(name="ph1x", bufs=1) as ph1x, \
             tc.tile_pool(name="pp1", bufs=3, space="PSUM") as pp1:
            xt = ph1x.tile([P, DT, n], F32R)
            wq = ph1x.tile([P, DT, 3 * D], F32R)
            xt_src = rr(xT_d.ap()).rearrange("p (dt n) -> p dt n", dt=DT)
            wq_src = rr(wqkvT_d.ap()).rearrange("p (dt e) -> p dt e", dt=DT)
            # big, wait-free transfers alternating over the two HWDGE rings
            h = DT // 2
            nc.sync.dma_start(xt[:, 0:h, :], xt_src[:, 0:h, :])
            nc.scalar.dma_start(xt[:, h:DT, :], xt_src[:, h:DT, :])
            for d in range(DT):
                (nc.sync if d % 2 == 0 else nc.scalar).dma_start(
                    wq[:, d, :], wq_src[:, d, :])

            # 1a: q,k transposed  (qkT[e-tile, :] = sum_d WqkvT[d, e].T @ xT[d, :])
            for E in range(2 * DT):
                for c in range(NCH):
                    ps = pp1.tile([P, CH], F32, tag="pp1")
                    for d in range(DT):
                        nc.tensor.matmul(ps, wq[:, d, E * P:(E + 1) * P],
                                         xt[:, d, c * CH:(c + 1) * CH],
                                         start=(d == 0), stop=(d == DT - 1))
                    nc.vector.tensor_copy(qk[:, E, c * CH:(c + 1) * CH], ps)

            # 1b: v natural, scattered into vaug's per-head 65-wide blocks
            for t in range(NT):
                for (cb, cw) in ((0, 512), (512, 256)):
                    psf = pp1.tile([P, 512], F32, tag="pp2", name="pp2")
                    ps = psf[:, :cw]
                    for d in range(DT):
                        nc.tensor.matmul(ps, xt[:, d, t * P:(t + 1) * P],
                                         wq[:, d, 2 * D + cb:2 * D + cb + cw],
                                         start=(d == 0), stop=(d == DT - 1))
                    h0 = cb // HD
                    nc.vector.tensor_copy(
                        vaug[:, t, h0:h0 + cw // HD, 0:HD],
                        ps.rearrange("p (h d) -> p h d", d=HD))

        # ---------------- phases 2+3: attention ----------------
        # Flat software pipeline over (pair, seq-tile, head) groups: the PE
        # queue is strict FIFO, so P@V matmuls (which wait on exp) are emitted
        # LAG groups behind the score matmuls.  At pair boundaries the next
        # pair's scores fill what was a ~4us PE bubble (which re-throttled the
        # HAM clock to 1.2 GHz every pair).
        with tc.tile_pool(name="pP", bufs=1) as pP, \
             tc.tile_pool(name="znorm", bufs=2) as znorm, \
             tc.tile_pool(name="tmpp", bufs=1) as tmpp, \
             tc.tile_pool(name="stps", bufs=2, space="PSUM") as stps, \
             tc.tile_pool(name="otps", bufs=2, space="PSUM") as otps:
            pstate = {}

            def pair_tiles(pr):
                if pr not in pstate:
                    pstate[pr] = (
                        (pP.tile([P, NT, n], F32R, tag="pa", name="pa"),
                         pP.tile([P, NT, n], F32R, tag="pb", name="pb")),
                        (otps.tile([HD + 1, n], F32, tag="ot", name="ot"),
                         otps.tile([HD + 1, n], F32, tag="ot", name="ot")))
                return pstate[pr]

            def emit_st(pr, t, hi):
                pboth, _ = pair_tiles(pr)
                lo = hi * HD
                st = stps.tile([P, n], F32, tag="st", name="st")
                for c in range(NCH):
                    nc.tensor.matmul(
                        st[:, c * CH:(c + 1) * CH],
                        qk[lo:lo + HD, DT + pr, t * P:(t + 1) * P],
                        qk[lo:lo + HD, pr, c * CH:(c + 1) * CH],
                        start=True, stop=True)
                nc.scalar.activation(pboth[hi][:, t, :], st, AF.Exp,
                                     bias=mb[:, t:t + 1], scale=SCALE)
                nc.vector.tensor_add(pboth[hi][:, t, t * P:(t + 1) * P],
                                     pboth[hi][:, t, t * P:(t + 1) * P],
                                     dtl[:, t, :])

            def emit_ot(pr, t, hi):
                pboth, ots = pair_tiles(pr)
                h = 2 * pr + hi
                for c in range(NCH):
                    sl = slice(c * CH, (c + 1) * CH)
                    nc.tensor.matmul(ots[hi][:, sl], vaug[:, t, h, :],
                                     pboth[hi][:, t, sl],
                                     start=(t == 0), stop=(t == NT - 1),
                                     skip_group_check=True)

            def emit_norm(pr):
                _, ots = pair_tiles(pr)
                for hi in range(2):
                    ot = ots[hi]
                    z65 = znorm.tile([HD + 1, n], F32, tag="z65")
                    nc.vector.tensor_copy(z65[HD:HD + 1, :], ot[HD:HD + 1, :])
                    nc.sync.dma_start(z65[0:1, :], z65[HD:HD + 1, :])  # Z to base 0
                    rbs = znorm.tile([HD, n], F32, tag="rbs")
                    nc.gpsimd.partition_broadcast(rbs, z65[0:1, :], channels=HD)
                    nc.vector.reciprocal_approx_fast(rbs, rbs)
                    if hi == 0:
                        nc.vector.tensor_mul(otn[0:HD, pr, :], ot[0:HD, :], rbs)
                    else:
                        tmp = tmpp.tile([HD, n], F32R, tag="tmp")
                        nc.vector.tensor_mul(tmp, ot[0:HD, :], rbs)
                        nc.sync.dma_start(otn[HD:P, pr, :], tmp)
                del pstate[pr]

            groups = [(pr, t, hi)
                      for pr in range(DT) for t in range(NT) for hi in range(2)]
            LAG = 3
            for i, g in enumerate(groups):
                emit_st(*g)
                if i >= LAG:
                    gj = groups[i - LAG]
                    emit_ot(*gj)
                    if gj[1] == NT - 1 and gj[2] == 1:
                        emit_norm(gj[0])
            for j in range(len(groups) - LAG, len(groups)):
                gj = groups[j]
                emit_ot(*gj)
                if gj[1] == NT - 1 and gj[2] == 1:
                    emit_norm(gj[0])

        # ---------------- phase 4: output projection ----------------
        with tc.tile_pool(name="ph4w", bufs=1) as ph4w, \
             tc.tile_pool(name="ob", bufs=3) as obp, \
             tc.tile_pool(name="p4", bufs=3, space="PSUM") as p4p:
            wpj = ph4w.tile([P, DT, D], F32R)
            wpj_src = rr(wprojT_d.ap()).rearrange("p (dt e) -> p dt e", dt=DT)
            nc.sync.dma_start(wpj[:, 0:3, :], wpj_src[:, 0:3, :])
            nc.sync.dma_start(wpj[:, 3:DT, :], wpj_src[:, 3:DT, :])
            for t in range(NT):
                ob = obp.tile([P, D], F32, tag="ob")
                for (cb, cw) in ((0, 512), (512, 256)):
                    psf = p4p.tile([P, 512], F32, tag="p4", name="p4")
                    ps = psf[:, :cw]
                    for d in range(DT):
                        nc.tensor.matmul(ps, otn[:, d, t * P:(t + 1) * P],
                                         wpj[:, d, cb:cb + cw],
                                         start=(d == 0), stop=False)
                    nc.tensor.matmul(ps, ones, bpj[:, cb:cb + cw],
                                     start=False, stop=True)
                    nc.vector.tensor_copy(ob[:, cb:cb + cw], ps)
                nc.sync.dma_start(out_d.ap()[t * P:(t + 1) * P, :], ob)

    nc.compile()
    return nc


def make_in_maps(x, mask, Wqkv, Wproj, bproj):
    x = np.ascontiguousarray(np.asarray(x), dtype=np.float32)
    mask = np.asarray(mask)
    def pack(wt):   # [D, cols] -> [128, DT*cols], row p = concat_d wt[d*128+p]
        cols = wt.shape[1]
        return np.ascontiguousarray(
            wt.reshape(DT, P, cols).transpose(1, 0, 2).reshape(P, DT * cols))
    wqkvT = pack(np.asarray(Wqkv, dtype=np.float32).T.copy())
    wprojT = pack(np.asarray(Wproj, dtype=np.float32).T.copy())
    bp = np.ascontiguousarray(np.asarray(bproj, dtype=np.float32).reshape(1, D))
    onesv = np.ones((1, P), dtype=np.float32)
    b, n, _ = x.shape
    nt = n // P
    in_maps = []
    for i in range(b):
        mf = mask[i].astype(np.float32)
        mcol = mf.reshape(nt, P).T.copy()              # [P, NT]
        in_maps.append({
            "xT": pack(np.ascontiguousarray(x[i].T)),
            "wqkvT": wqkvT,
            "wprojT": wprojT,
            "bproj": bp,
            "mbias": np.ascontiguousarray((mcol - 1.0) * (-NEGMASK)),
            "omm": np.ascontiguousarray((1.0 - mcol) * BIGG),
            "onesv": onesv,
        })
    return in_maps


_NC_CACHE = {}


def get_nc(n=N):
    if n not in _NC_CACHE:
        _NC_CACHE[n] = build_nc(n)
    return _NC_CACHE[n]


def kernel(x, mask, Wqkv, Wproj, bproj):
    x = np.asarray(x)
    b, n, _ = x.shape
    nc = get_nc(n)
    in_maps = make_in_maps(x, mask, Wqkv, Wproj, bproj)
    res = bass_utils.run_bass_kernel_spmd(nc, in_maps, core_ids=list(range(b)))
    out = np.stack([res.results[i]["out"] for i in range(b)], axis=0)
    return out.astype(np.float32)



# revision 9
# speedup vs baseline: 1.2288x; 1.2288x over previous
"""Masked multi-head attention block (B=8, N=1024, D=768, H=12) on 8 NeuronCores.

Strategy: pure data-parallel over batch (1 batch element per core).  Per core,
one fully-fused software pipeline keeps the PE array busy (and its HAM clock
gate warm) end to end:

  prologue:  need-ordered chunked input DMA across the three DMA queues;
             first matmul issues ~6us in.
  v-phase:   v[n, e] = xT(lhsT) @ WvT, stored bf16 in per-head-pair blocks
             [v_even | ones64][ones64 | v_odd]: the 64 ones columns replicate
             the softmax denominator Z across 64 PSUM partitions for free.
  qk(pr):    q,k e-tiles of head-pair pr (f32r matmuls, cast to bf16).
             Pairs 0,1 run before the attention loop; pair pr+2 is emitted as
             PE *filler* inside attention pair pr, exactly absorbing the PE
             slack of the ACT-bound softmax loop (keeps HAM at 2.4 GHz).
  attention: per (pair, hi, t): ST = kT(lhsT) @ qT (bf16, 1024-wide moving),
             P = exp(ST*scale + key_mask_bias) on ACT (PSUM -> SBUF bf16),
             diag add (padded queries -> one-hot), P@Vaug accumulated over t
             (PV on one 64-partition half, Z replicated on the other), then
             norm: recip(Z-half) -> tiny partition-shift DMA -> mul into otn.
  proj:      out[n, e] = otn(lhsT) @ WprojT + ones(K=1) x bproj, accumulated
             in PSUM and DMA'd straight PSUM -> DRAM (no SBUF staging).

Numerics: QKV/proj contractions (K=768) in float32r; q/k/v/P in bf16 (errors
~1e-3 absolute vs a ~6.0 absmax reference; tolerance is 2e-2 relative).
"""
import sys
for _p in ('/opt/trn_rl_repo',):
    if _p not in sys.path:
        sys.path.insert(0, _p)

from contextlib import ExitStack

import numpy as np

import concourse.bass as bass
import concourse.bacc as bacc
import concourse.mybir as mybir
import concourse.tile as tile
from concourse import bass_utils

F32 = mybir.dt.float32
F32R = mybir.dt.float32r
BF16 = mybir.dt.bfloat16
AF = mybir.ActivationFunctionType

B, N, D, H, HD = 8, 1024, 768, 12, 64
P = 128
DT = D // P            # 6 d-tiles
PAIRS = H // 2         # 6 head pairs (== DT)
SCALE = HD ** -0.5
NEGMASK = -30000.0     # exp(x + NEGMASK) == 0.0 in fp32 for any realistic score
BIGG = 1e15            # diagonal dominance constant for padded-query rows


def build_nc(n=N, debug=False):
    NT = n // P                    # seq tiles (8)
    nc = bacc.Bacc("TRN2", target_bir_lowering=False, debug=False)

    xT_d = nc.dram_tensor("xT", [P, DT * n], F32, kind="ExternalInput")
    wqkT_d = nc.dram_tensor("wqkT", [P, DT * 2 * D], F32, kind="ExternalInput")
    wvT_d = nc.dram_tensor("wvT", [P, DT * D], F32, kind="ExternalInput")
    wprojT_d = nc.dram_tensor("wprojT", [P, DT * D], F32, kind="ExternalInput")
    bproj_d = nc.dram_tensor("bproj", [1, D], F32, kind="ExternalInput")
    mbias_d = nc.dram_tensor("mbias", [P, NT], F32, kind="ExternalInput")
    omm_d = nc.dram_tensor("omm", [P, NT], F32, kind="ExternalInput")
    ones_d = nc.dram_tensor("onesv", [1, P], F32, kind="ExternalInput")
    out_d = nc.dram_tensor("out", [n, D], F32, kind="ExternalOutput")

    def rr(ap):
        return ap.bitcast(F32R)

    with tile.TileContext(nc) as tc, ExitStack() as ctx:
        persist = ctx.enter_context(tc.tile_pool(name="persist", bufs=1))
        xt = persist.tile([P, DT, n], F32R)            # 24KB/part
        wq = persist.tile([P, DT, 2 * D], F32R)        # 36KB
        wpj = persist.tile([P, DT, D], F32R)           # 18KB
        vaug = persist.tile([P, NT, PAIRS, 2 * P], BF16)  # 24KB
        qks = persist.tile([P, 3, 2, n], BF16)         # 12KB (pair slots: q,k)
        otn = persist.tile([P, DT, n], F32R)           # 24KB
        pa = persist.tile([P, NT, n], BF16)            # 16KB
        pb = persist.tile([P, NT, n], BF16)            # 16KB
        dtl = persist.tile([P, NT, P], BF16)           # 2KB
        mb = persist.tile([P, NT], F32)
        om = persist.tile([P, NT], F32)
        ones = persist.tile([1, P], F32R)
        bpj = persist.tile([1, D], F32R)

        # ---------------- input DMAs, need-ordered across 3 queues ---------
        xt_src = rr(xT_d.ap()).rearrange("p (dt n) -> p dt n", dt=DT)
        wqk_src = rr(wqkT_d.ap()).rearrange("p (dt e) -> p dt e", dt=DT)
        wv_src = rr(wvT_d.ap()).rearrange("p (dt e) -> p dt e", dt=DT)
        wpj_src = rr(wprojT_d.ap()).rearrange("p (dt e) -> p dt e", dt=DT)

        # gpsimd queue: small tensors, then the (late-needed) proj weights
        nc.gpsimd.dma_start(mb, mbias_d.ap())
        nc.gpsimd.dma_start(om, omm_d.ap())
        nc.gpsimd.dma_start(ones, rr(ones_d.ap()))
        nc.gpsimd.dma_start(bpj, rr(bproj_d.ap()))
        nc.gpsimd.dma_start(wpj, wpj_src)

        # sync queue: x by seq-tile (v-phase consumes in t order), then
        # q/k weights for pairs 2..5 (filler work, needed progressively)
        for t in range(NT):
            nc.sync.dma_start(xt[:, :, t * P:(t + 1) * P],
                              xt_src[:, :, t * P:(t + 1) * P])
        for pr in range(2, PAIRS):
            nc.sync.dma_start(wq[:, :, pr * P:(pr + 1) * P],
                              wqk_src[:, :, pr * P:(pr + 1) * P])
            nc.sync.dma_start(wq[:, :, D + pr * P:D + (pr + 1) * P],
                              wqk_src[:, :, D + pr * P:D + (pr + 1) * P])

        # scalar queue: v weights (needed first), then q/k for pairs 0,1
        with tc.tile_pool(name="wvpool", bufs=1) as wvp, \
             tc.tile_pool(name="scratch", bufs=1) as scr:
            wv = wvp.tile([P, DT, D], F32R)            # 18KB, freed after v
            dtf = scr.tile([P, NT, P], F32)            # 4KB, prologue only
            warm = scr.tile([1, 1], F32)
            nc.scalar.dma_start(wv[:, :, 0:512], wv_src[:, :, 0:512])
            nc.scalar.dma_start(wv[:, :, 512:D], wv_src[:, :, 512:D])
            for pr in range(2):
                nc.scalar.dma_start(wq[:, :, pr * P:(pr + 1) * P],
                                    wqk_src[:, :, pr * P:(pr + 1) * P])
                nc.scalar.dma_start(wq[:, :, D + pr * P:D + (pr + 1) * P],
                                    wqk_src[:, :, D + pr * P:D + (pr + 1) * P])

            # prologue compute: ones blocks of vaug, diag tiles, ACT warmup
            nc.vector.memset(vaug[:, :, :, HD:3 * HD], 1.0)
            for t in range(NT):
                nc.gpsimd.affine_select(
                    out=dtf[:, t, :],
                    in_=om[:, t:t + 1].to_broadcast((P, P)),
                    pattern=[[-1, P]],
                    compare_op=mybir.AluOpType.is_equal,
                    fill=0.0, base=0, channel_multiplier=1,
                )
            nc.vector.tensor_copy(dtl, dtf)
            # load the exp table set during the v-phase, not at first score
            nc.scalar.activation(warm, mb[0:1, 0:1], AF.Exp, bias=0.0, scale=1.0)

            with tc.tile_pool(name="stps", bufs=2, space="PSUM") as stps, \
                 tc.tile_pool(name="otps", bufs=2, space="PSUM") as otps, \
                 tc.tile_pool(name="rbsp", bufs=2) as rbsp:

                # ---------------- v-phase ----------------
                # per t: 12 accumulating MMs into one [P, 768] PSUM region,
                # then strided copies into vaug's pair blocks (even heads at
                # block cols 0:64, odd heads at 192:256).
                for t in range(NT):
                    vp = otps.tile([P, n], F32, tag="ot", name="vp")
                    for (cb, cw) in ((0, 512), (512, 256)):
                        for d in range(DT):
                            nc.tensor.matmul(
                                vp[:, cb:cb + cw],
                                xt[:, d, t * P:(t + 1) * P],
                                wv[:, d, cb:cb + cw],
                                start=(d == 0), stop=(d == DT - 1))
                    vsp = vp[:, 0:D].rearrange("p (pr h d) -> p pr h d",
                                               h=2, d=HD)
                    # evens on ScE, odds on DVE (both engines idle here)
                    nc.scalar.copy(vaug[:, t, :, 0:HD], vsp[:, :, 0, :])
                    nc.vector.tensor_copy(vaug[:, t, :, 3 * HD:4 * HD],
                                          vsp[:, :, 1, :])

                # ---------------- qk units + attention pipeline ----------
                def emit_qk_quarter(pr, j, c):
                    """One c-half of q (j=0) or k (j=1) for pair pr: 6 MMs."""
                    key = (pr, j)
                    if key not in qk_ps:
                        qk_ps[key] = stps.tile([P, n], F32, tag="st",
                                               name="qkp")
                    ps = qk_ps[key]
                    off = j * D + pr * P
                    for d in range(DT):
                        nc.tensor.matmul(
                            ps[:, c * 512:(c + 1) * 512],
                            wq[:, d, off:off + P],
                            xt[:, d, c * 512:(c + 1) * 512],
                            start=(d == 0), stop=(d == DT - 1))
                    if c == 1:
                        nc.vector.tensor_copy(qks[:, pr % 3, j, :], ps)
                        del qk_ps[key]

                qk_ps = {}
                for pr in range(2):
                    for j in range(2):
                        for c in range(2):
                            emit_qk_quarter(pr, j, c)

                ot_tiles = {}

                def emit_st(pr, hi, t):
                    slot = pr % 3
                    lo = hi * HD
                    st = stps.tile([P, n], F32, tag="st", name="st")
                    for c in range(2):
                        sl = slice(c * 512, (c + 1) * 512)
                        nc.tensor.matmul(
                            st[:, sl],
                            qks[lo:lo + HD, slot, 1, t * P:(t + 1) * P],
                            qks[lo:lo + HD, slot, 0, sl],
                            start=True, stop=True)
                    px = pa if hi == 0 else pb
                    nc.scalar.activation(px[:, t, :], st, AF.Exp,
                                         bias=mb[:, t:t + 1], scale=SCALE)
                    nc.vector.tensor_add(px[:, t, t * P:(t + 1) * P],
                                         px[:, t, t * P:(t + 1) * P],
                                         dtl[:, t, :])

                def emit_pv(pr, hi, t):
                    if (pr, hi) not in ot_tiles:
                        ot_tiles[(pr, hi)] = otps.tile([P, n], F32, tag="ot",
                                                       name="ot")
                    ot = ot_tiles[(pr, hi)]
                    px = pa if hi == 0 else pb
                    for c in range(2):
                        sl = slice(c * 512, (c + 1) * 512)
                        nc.tensor.matmul(
                            ot[:, sl], vaug[:, t, pr, hi * P:(hi + 1) * P],
                            px[:, t, sl],
                            start=(t == 0), stop=(t == NT - 1),
                            skip_group_check=True)

                def emit_norm(pr, hi):
                    # reciprocal_approx_fast only works at partition base 0
                    # (custom-DVE op); stage Z to base 0 where needed.
                    ot = ot_tiles.pop((pr, hi))
                    if hi == 0:      # PV on 0:64, Z replicated on 64:128
                        zst = rbsp.tile([P, n], F32, tag="rbs", name="zst")
                        rbs = rbsp.tile([P, n], F32, tag="rbs", name="rbs")
                        nc.vector.tensor_copy(zst[HD:P, :], ot[HD:P, :])
                        nc.gpsimd.dma_start(zst[0:HD, :], zst[HD:P, :])
                        nc.vector.reciprocal_approx_fast(rbs[0:HD, :],
                                                         zst[0:HD, :])
                        nc.vector.tensor_mul(otn[0:HD, pr, :], ot[0:HD, :],
                                             rbs[0:HD, :])
                    else:            # Z replicated on 0:64, PV on 64:128
                        rbs = rbsp.tile([P, n], F32, tag="rbs", name="rbs")
                        nc.vector.reciprocal_approx_fast(rbs[0:HD, :],
                                                         ot[0:HD, :])
                        nc.gpsimd.dma_start(rbs[HD:P, :], rbs[0:HD, :])
                        nc.vector.tensor_mul(otn[HD:P, pr, :], ot[HD:P, :],
                                             rbs[HD:P, :])

                groups = [(pr, hi, t)
                          for pr in range(PAIRS)
                          for hi in range(2) for t in range(NT)]
                LAG = 2
                for i, g in enumerate(groups):
                    emit_st(*g)
                    pr, hi, t = g
                    # filler: qk projection for pair pr+2 (absorbs PE slack;
                    # keeps HAM warm).  Quarter-units at adjacent groups so a
                    # held accumulation spans at most one 'st' ring slot.
                    if pr + 2 < PAIRS and t in (2, 3):
                        emit_qk_quarter(pr + 2, hi, t - 2)
                    if i >= LAG:
                        gj = groups[i - LAG]
                        emit_pv(*gj)
                        if gj[2] == NT - 1:
                            emit_norm(gj[0], gj[1])
                for i in range(len(groups) - LAG, len(groups)):
                    gj = groups[i]
                    emit_pv(*gj)
                    if gj[2] == NT - 1:
                        emit_norm(gj[0], gj[1])

        # ---------------- output projection ----------------
        with tc.tile_pool(name="pjp", bufs=3, space="PSUM") as pjp, \
             tc.tile_pool(name="obp", bufs=3) as obp:
            for t in range(NT):
                ps = pjp.tile([P, D], F32, tag="pj")
                ob = obp.tile([P, D], F32, tag="ob")
                for (cb, cw) in ((0, 512), (512, 256)):
                    for d in range(DT):
                        nc.tensor.matmul(ps[:, cb:cb + cw],
                                         otn[:, d, t * P:(t + 1) * P],
                                         wpj[:, d, cb:cb + cw],
                                         start=(d == 0), stop=False)
                    nc.tensor.matmul(ps[:, cb:cb + cw], ones,
                                     bpj[:, cb:cb + cw],
                                     start=False, stop=True)
                # ACT and DVE are both idle post-attention: split the drain
                nc.scalar.copy(ob[:, 0:512], ps[:, 0:512])
                nc.vector.tensor_copy(ob[:, 512:D], ps[:, 512:D])
                nc.sync.dma_start(out_d.ap()[t * P:(t + 1) * P, :], ob)

    nc.compile()
    return nc


def make_in_maps(x, mask, Wqkv, Wproj, bproj):
    x = np.ascontiguousarray(np.asarray(x), dtype=np.float32)
    mask = np.asarray(mask)
    def pack(wt):   # [D, cols] -> [128, DT*cols], row p = concat_d wt[d*128+p]
        cols = wt.shape[1]
        return np.ascontiguousarray(
            wt.reshape(DT, P, cols).transpose(1, 0, 2).reshape(P, DT * cols))
    wqkvT = np.asarray(Wqkv, dtype=np.float32).T.copy()   # [D, 3D]
    wqkT = pack(wqkvT[:, 0:2 * D])
    wvT = pack(wqkvT[:, 2 * D:3 * D])
    wprojT = pack(np.asarray(Wproj, dtype=np.float32).T.copy())
    bp = np.ascontiguousarray(np.asarray(bproj, dtype=np.float32).reshape(1, D))
    onesv = np.ones((1, P), dtype=np.float32)
    b, n, _ = x.shape
    nt = n // P
    in_maps = []
    for i in range(b):
        mf = mask[i].astype(np.float32)
        mcol = mf.reshape(nt, P).T.copy()              # [P, NT]
        in_maps.append({
            "xT": pack(np.ascontiguousarray(x[i].T)),
            "wqkT": wqkT,
            "wvT": wvT,
            "wprojT": wprojT,
            "bproj": bp,
            "mbias": np.ascontiguousarray((mcol - 1.0) * (-NEGMASK)),
            "omm": np.ascontiguousarray((1.0 - mcol) * BIGG),
            "onesv": onesv,
        })
    return in_maps


_NC_CACHE = {}


def get_nc(n=N):
    if n not in _NC_CACHE:
        _NC_CACHE[n] = build_nc(n)
    return _NC_CACHE[n]


def kernel(x, mask, Wqkv, Wproj, bproj):
    x = np.asarray(x)
    b, n, _ = x.shape
    nc = get_nc(n)
    in_maps = make_in_maps(x, mask, Wqkv, Wproj, bproj)
    res = bass_utils.run_bass_kernel_spmd(nc, in_maps, core_ids=list(range(b)))
    out = np.stack([res.results[i]["out"] for i in range(b)], axis=0)
    return out.astype(np.float32)
